# revision 9
# baseline (speedup 1.0000x reference)
"""GAT layer (nn_GATLayer) on 8 Trainium2 NeuronCores.

Math (reference):
    Wh = X @ weight                      [N, F]
    s  = Wh @ a[:F];  t = Wh @ a[F:]     [N, 1]
    e  = relu(s_i + t_j)                 [N, N]
    att = softmax(where(A > 0, e, -9e15), axis=1)
    out = elu(att @ Wh)

Kernel formulation (shift-free softmax, exact up to fp rounding):
    p_ij  = A_ij * max(exp(s_i + t_j), 1)     (exp(relu(x)) = max(exp(x), 1))
    out_i = elu((p_i: @ Wh) / sum_j p_ij)

Sharding: 1D row partition of A across 8 cores (1024 rows each); X,
weight, a replicated; out rows gathered on host.

Per-core dataflow (v2: transposed orientation [j, i]; the only large
transpose is A's, as 64 big DRAM->SBUF xbar DMAs):
  - A_blk int32 -> bf16 {0,1.0} via SWDGE DRAM->DRAM cast (8 chunks),
    then 64 DmaTranspose [1024, 128] -> at_slab [128 j, 1024 i].
  - X -> bf16 DRAM (D2D cast), 4 big transposes -> X^T chunks; Wh_nat
    [128 j, 128 f] + t columns from PE (stationary = X^T slice, moving
    = [weight | w_t]); w_t = weight.T-chunks @ a via tiny PE matmuls.
  - s (own rows) from an X_own mini-matmul, assembled into a DRAM row,
    broadcast-cast back as S_bcast [128, 1024 i].
  - main loop over 64 j-tiles: ACT z = exp(S_bcast + t_j); one fused
    DVE op p^T = (z max 1) * at_slab; PE: psum_oT [128 f, 1024 i] +=
    contraction of Wh_nat[jt] with p^T (N=512 x2), denominator row via
    ones stationary.
  - epilogue: reciprocal of denom -> DRAM broadcast -> scale, ELU
    (exp(min(x,0)) - 1 + max(x,0)), 8 PE transposes back to natural,
    DMA out.
"""

import numpy as np

import concourse.bass as bass
import concourse.bacc as bacc
import concourse.mybir as mybir
import concourse.tile as tile
from concourse.bass_utils import run_bass_kernel_spmd

N = 8192
F_IN = 512
F_OUT = 128
N_CORES = 8
ROWS = N // N_CORES          # 1024 rows per core
RT = ROWS // 128             # 8 own row tiles
NT = N // 128                # 64 j tiles
KC = F_IN // 128             # 4 f_in chunks
DCH = 8                      # A cast D2D chunks

FP32 = mybir.dt.float32
BF16 = mybir.dt.bfloat16
Alu = mybir.AluOpType
Act = mybir.ActivationFunctionType

_cache = {}


def _build(repeat=1):
    nc = bacc.Bacc("TRN2", target_bir_lowering=False, debug=False,
                   num_devices=N_CORES)

    A_blk = nc.dram_tensor("A_blk", [ROWS, N], mybir.dt.int32, kind="ExternalInput")
    X_own = nc.dram_tensor("X_own", [ROWS, F_IN], FP32, kind="ExternalInput")
    weight = nc.dram_tensor("weight", [F_IN, F_OUT], FP32, kind="ExternalInput")
    a_vec = nc.dram_tensor("a_vec", [2 * F_OUT, 1], FP32, kind="ExternalInput")
    ident = nc.dram_tensor("ident", [128, 128], FP32, kind="ExternalInput")
    out_d = nc.dram_tensor("out", [ROWS, F_OUT], FP32, kind="ExternalOutput")

    with tile.TileContext(nc) as tc:
        for rep in range(repeat):
            _body(nc, tc, rep, A_blk, X_own, weight, a_vec, ident, out_d)

    nc.compile()
    return nc


def _body(nc, tc, rep, A_blk, X_own, weight, a_vec, ident, out_d):
    with tc.tile_pool(name=f"dram{rep}", bufs=1, space="DRAM") as dram_pool:
            A_bf = dram_pool.tile([ROWS, N], BF16)
            Xo_bf = dram_pool.tile([ROWS, F_IN], BF16)
            s_dram = dram_pool.tile([1, ROWS], FP32)
            r_dram = dram_pool.tile([1, ROWS], FP32)
            cc_in_wh = nc.dram_tensor(f"cc_in_wh{rep}", [ROWS, F_OUT], BF16)
            cc_out_wh = nc.dram_tensor(
                f"cc_out_wh{rep}", [N, F_OUT], BF16, addr_space="Shared"
            )
            cc_in_t = nc.dram_tensor(f"cc_in_t{rep}", [RT, 128], FP32)
            cc_out_t = nc.dram_tensor(
                f"cc_out_t{rep}", [NT, 128], FP32, addr_space="Shared"
            )

            # ---- D2D casts (SWDGE): A int32 -> bf16, X f32 -> bf16 ----
            for c in range(DCH):
                w = N // DCH
                nc.gpsimd.dma_start(
                    out=A_bf[:, w * c : w * (c + 1)],
                    in_=A_blk[:, w * c : w * (c + 1)],
                )
            nc.gpsimd.dma_start(out=Xo_bf[:, :], in_=X_own[:, :])

            with (
                tc.tile_pool(name=f"setup{rep}", bufs=1) as setup,
                tc.tile_pool(name=f"whn{rep}", bufs=NT) as whn_pool,
                tc.tile_pool(name=f"slab{rep}", bufs=6) as slab_pool,
                tc.tile_pool(name=f"zz{rep}", bufs=3) as zz_pool,
                tc.tile_pool(name=f"pp{rep}", bufs=3) as pp_pool,
                tc.tile_pool(name=f"epi{rep}", bufs=2) as epi_pool,
                tc.tile_pool(name=f"psA{rep}", bufs=2, space="PSUM") as psA,
            ):
                # ---------------- setup ----------------
                idn = setup.tile([128, 128], FP32)
                nc.sync.dma_start(out=idn, in_=ident[:, :])
                ones_c = setup.tile([128, 1], BF16)
                nc.vector.memset(ones_c, 1.0)

                a_cat = setup.tile([128, 2], BF16)
                nc.gpsimd.dma_start(out=a_cat[:, 0:1], in_=a_vec[0:F_OUT, :])
                nc.gpsimd.dma_start(out=a_cat[:, 1:2], in_=a_vec[F_OUT:, :])

                # w_all[k] = [weight_k bf16 | w_t_k | w_s_k]  [128, 130]
                w_all = []
                for k in range(KC):
                    wa = setup.tile([128, F_OUT + 2], BF16, tag=f"w_all{k}")
                    nc.gpsimd.dma_start(
                        out=wa[:, 0:F_OUT], in_=weight[128 * k : 128 * (k + 1), :]
                    )
                    w_all.append(wa)
                for k in range(KC):
                    wT = setup.tile([128, 128], BF16, tag=f"wT{k}")
                    nc.sync.dma_start(
                        out=wT, in_=w_all[k][:, 0:F_OUT], transpose=True
                    )
                    ps = psA.tile([128, 2], FP32, tag="ps")
                    nc.tensor.matmul(ps, wT, a_cat, start=True, stop=True)
                    # col F_OUT = w_t (a[F:]), col F_OUT+1 = w_s (a[:F])
                    nc.vector.tensor_copy(
                        w_all[k][:, F_OUT : F_OUT + 1], ps[:, 1:2]
                    )
                    nc.vector.tensor_copy(
                        w_all[k][:, F_OUT + 1 : F_OUT + 2], ps[:, 0:1]
                    )

                # ---------------- X_own^T chunks ----------------
                xoT = []
                for k in range(KC):
                    xt = setup.tile([128, ROWS], BF16, tag=f"xoT{k}")
                    nc.sync.dma_start(
                        out=xt, in_=Xo_bf[:, 128 * k : 128 * (k + 1)], transpose=True
                    )
                    xoT.append(xt)

                # ------- own rows: [Wh | t | s] = Xo^T.T-contract @ w_all -------
                s_cols = setup.tile([128, RT], FP32)
                t_own = setup.tile([128, RT], FP32)
                for q in range(RT):
                    ps = psA.tile([128, F_OUT + 2], FP32, tag="ps")
                    for k in range(KC):
                        nc.tensor.matmul(
                            ps,
                            xoT[k][:, 128 * q : 128 * (q + 1)],
                            w_all[k],
                            start=(k == 0),
                            stop=(k == KC - 1),
                        )
                    wh = epi_pool.tile([128, F_OUT], BF16, tag="wh_own")
                    nc.vector.tensor_copy(wh, ps[:, 0:F_OUT])
                    nc.sync.dma_start(
                        out=cc_in_wh[128 * q : 128 * (q + 1), :], in_=wh
                    )
                    nc.vector.tensor_copy(
                        t_own[:, q : q + 1], ps[:, F_OUT : F_OUT + 1]
                    )
                    nc.vector.tensor_copy(
                        s_cols[:, q : q + 1], ps[:, F_OUT + 1 : F_OUT + 2]
                    )
                # s -> DRAM row -> broadcast
                ps_sT = psA.tile([RT, 128], FP32, tag="ps")
                nc.tensor.transpose(ps_sT, s_cols, idn)
                sT = setup.tile([RT, 128], FP32)
                nc.vector.tensor_copy(sT, ps_sT)
                nc.sync.dma_start(out=s_dram[:, :], in_=sT)
                s_bc = setup.tile([128, ROWS], FP32)
                nc.gpsimd.dma_start(
                    out=s_bc,
                    in_=bass.AP(
                        tensor=s_dram.tensor, offset=s_dram.offset,
                        ap=[[0, 128], [1, ROWS]],
                    ),
                )
                # t-own -> [RT, 128] tile-major -> gather
                ps_tT = psA.tile([RT, 128], FP32, tag="ps")
                nc.tensor.transpose(ps_tT, t_own, idn)
                tT = setup.tile([RT, 128], FP32)
                nc.vector.tensor_copy(tT, ps_tT)
                nc.sync.dma_start(out=cc_in_t[:, :], in_=tT)

                # ------- AllGather Wh + t across cores -------
                nc.gpsimd.collective_compute(
                    "AllGather", Alu.bypass,
                    replica_groups=[list(range(N_CORES))],
                    ins=[cc_in_wh[:, :]], outs=[cc_out_wh[:, :]],
                )
                nc.gpsimd.collective_compute(
                    "AllGather", Alu.bypass,
                    replica_groups=[list(range(N_CORES))],
                    ins=[cc_in_t[:, :]], outs=[cc_out_t[:, :]],
                )
                wh_nat = []
                for r in range(NT):
                    wh = whn_pool.tile([128, F_OUT], BF16)
                    nc.sync.dma_start(
                        out=wh, in_=cc_out_wh[128 * r : 128 * (r + 1), :]
                    )
                    wh_nat.append(wh)
                tg = setup.tile([NT, 128], FP32)
                nc.sync.dma_start(out=tg, in_=cc_out_t[:, :])
                ps_tc = psA.tile([128, NT], FP32, tag="ps")
                nc.tensor.transpose(ps_tc, tg, idn[0:NT, 0:NT])
                t_cols = setup.tile([128, NT], FP32)
                nc.vector.tensor_copy(t_cols, ps_tc)

                # ---------------- A^T slabs + main loop ----------------
                with (
                    tc.tile_pool(name=f"psO{rep}", bufs=1, space="PSUM") as psO,
                    tc.tile_pool(name=f"psD{rep}", bufs=1, space="PSUM") as psD,
                ):
                    ps_oT = psO.tile([128, ROWS], FP32)
                    ps_d = psD.tile([1, ROWS], FP32)
                    for jt in range(NT):
                        at = slab_pool.tile([128, ROWS], BF16)
                        nc.sync.dma_start(
                            out=at, in_=A_bf[:, 128 * jt : 128 * (jt + 1)],
                            transpose=True,
                        )
                        z = zz_pool.tile([128, ROWS], BF16)
                        nc.scalar.activation(
                            out=z, in_=s_bc, func=Act.Exp,
                            bias=t_cols[:, jt : jt + 1],
                        )
                        p = pp_pool.tile([128, ROWS], BF16)
                        nc.vector.scalar_tensor_tensor(
                            out=p, in0=z, scalar=1.0, in1=at,
                            op0=Alu.max, op1=Alu.mult,
                        )
                        first, last = jt == 0, jt == NT - 1
                        for h in range(2):
                            sl = slice(512 * h, 512 * (h + 1))
                            nc.tensor.matmul(
                                ps_oT[:, sl], wh_nat[jt], p[:, sl],
                                start=first, stop=last, skip_group_check=True,
                            )
                            nc.tensor.matmul(
                                ps_d[:, sl], ones_c, p[:, sl],
                                start=first, stop=last, skip_group_check=True,
                            )

                    # ---------------- epilogue ----------------
                    rec = epi_pool.tile([1, ROWS], FP32, tag="rec")
                    nc.vector.reciprocal(rec, ps_d)
                    nc.sync.dma_start(out=r_dram[:, :], in_=rec)
                    r_bc = epi_pool.tile([128, ROWS], FP32, tag="r_bc")
                    nc.gpsimd.dma_start(
                        out=r_bc,
                        in_=bass.AP(
                            tensor=r_dram.tensor, offset=r_dram.offset,
                            ap=[[0, 128], [1, ROWS]],
                        ),
                    )
                    xsc = epi_pool.tile([128, ROWS], FP32, tag="xsc")
                    nc.vector.tensor_tensor(
                        out=xsc, in0=ps_oT, in1=r_bc, op=Alu.mult
                    )
                    # ELU = exp(min(x,0)) - 1 + max(x,0)
                    m0 = epi_pool.tile([128, ROWS], FP32, tag="m0")
                    nc.vector.tensor_scalar(
                        out=m0, in0=xsc, scalar1=0.0, scalar2=None, op0=Alu.min
                    )
                    e0 = epi_pool.tile([128, ROWS], FP32, tag="e0")
                    nc.scalar.activation(out=e0, in_=m0, func=Act.Exp)
                    r0 = epi_pool.tile([128, ROWS], FP32, tag="r0")
                    nc.vector.tensor_scalar(
                        out=r0, in0=xsc, scalar1=0.0, scalar2=None, op0=Alu.max
                    )
                    oT = epi_pool.tile([128, ROWS], FP32, tag="oT")
                    nc.vector.scalar_tensor_tensor(
                        out=oT, in0=e0, scalar=-1.0, in1=r0,
                        op0=Alu.add, op1=Alu.add,
                    )
                    for q in range(RT):
                        ps_f = psA.tile([128, 128], FP32, tag="ps")
                        nc.tensor.transpose(
                            ps_f, oT[:, 128 * q : 128 * (q + 1)], idn
                        )
                        of = epi_pool.tile([128, F_OUT], FP32, tag="of")
                        nc.scalar.copy(of, ps_f)
                        nc.sync.dma_start(
                            out=out_d[128 * q : 128 * (q + 1), :], in_=of
                        )


def kernel(X, A, weight, a, _trace=False, _tmpdir=None):
    X = np.ascontiguousarray(np.asarray(X, dtype=np.float32))
    A = np.ascontiguousarray(np.asarray(A, dtype=np.int32))
    weight = np.ascontiguousarray(np.asarray(weight, dtype=np.float32))
    a = np.ascontiguousarray(np.asarray(a, dtype=np.float32))

    if "nc" not in _cache:
        _cache["nc"] = _build()
    nc = _cache["nc"]

    ident = np.eye(128, dtype=np.float32)
    in_maps = []
    for c in range(N_CORES):
        i0 = c * ROWS
        in_maps.append(
            {
                "A_blk": A[i0 : i0 + ROWS],
                "X_own": X[i0 : i0 + ROWS],
                "weight": weight,
                "a_vec": a,
                "ident": ident,
            }
        )

    res = run_bass_kernel_spmd(
        nc, in_maps, core_ids=list(range(N_CORES)), trace=_trace, tmpdir=_tmpdir
    )
    out = np.concatenate([res.results[c]["out"] for c in range(N_CORES)], axis=0)
    if _trace:
        kernel._last_results = res
    return out


# revision 12
# speedup vs baseline: 3.1383x; 3.1383x over previous
"""GAT layer (nn_GATLayer) on 8 Trainium2 NeuronCores.

Math (reference):
    Wh = X @ weight                      [N, F]
    s  = Wh @ a[:F];  t = Wh @ a[F:]     [N, 1]
    e  = relu(s_i + t_j)                 [N, N]
    att = softmax(where(A > 0, e, -9e15), axis=1)
    out = elu(att @ Wh)

Kernel formulation (shift-free softmax, exact up to fp rounding):
    p_ij  = A_ij * max(exp(s_i + t_j), 1)     (exp(relu(x)) = max(exp(x), 1))
    out_i = elu((p_i: @ Wh) / sum_j p_ij)

Sharding: 1D row partition of A across 8 cores (1024 rows each); X,
weight, a replicated; out rows gathered on host.

Per-core dataflow (v2: transposed orientation [j, i]; the only large
transpose is A's, as 64 big DRAM->SBUF xbar DMAs):
  - A_blk int32 -> bf16 {0,1.0} via SWDGE DRAM->DRAM cast (8 chunks),
    then 64 DmaTranspose [1024, 128] -> at_slab [128 j, 1024 i].
  - X -> bf16 DRAM (D2D cast), 4 big transposes -> X^T chunks; Wh_nat
    [128 j, 128 f] + t columns from PE (stationary = X^T slice, moving
    = [weight | w_t]); w_t = weight.T-chunks @ a via tiny PE matmuls.
  - s (own rows) from an X_own mini-matmul, assembled into a DRAM row,
    broadcast-cast back as S_bcast [128, 1024 i].
  - main loop over 64 j-tiles: ACT z = exp(S_bcast + t_j); one fused
    DVE op p^T = (z max 1) * at_slab; PE: psum_oT [128 f, 1024 i] +=
    contraction of Wh_nat[jt] with p^T (N=512 x2), denominator row via
    ones stationary.
  - epilogue: reciprocal of denom -> DRAM broadcast -> scale, ELU
    (exp(min(x,0)) - 1 + max(x,0)), 8 PE transposes back to natural,
    DMA out.
"""

import numpy as np

import concourse.bass as bass
import concourse.bacc as bacc
import concourse.mybir as mybir
import concourse.tile as tile
from concourse.bass_utils import run_bass_kernel_spmd

N = 8192
F_IN = 512
F_OUT = 128
N_CORES = 8
ROWS = N // N_CORES          # 1024 rows per core
RT = ROWS // 128             # 8 own row tiles
NT = N // 128                # 64 j tiles
KC = F_IN // 128             # 4 f_in chunks
DCH = 8                      # A cast D2D chunks

FP32 = mybir.dt.float32
BF16 = mybir.dt.bfloat16
Alu = mybir.AluOpType
Act = mybir.ActivationFunctionType

_cache = {}


def _build(repeat=1):
    nc = bacc.Bacc("TRN2", target_bir_lowering=False, debug=False,
                   num_devices=N_CORES)

    A_blk = nc.dram_tensor("A_blk", [ROWS, N], mybir.dt.int32, kind="ExternalInput")
    X_own = nc.dram_tensor("X_own", [ROWS, F_IN], FP32, kind="ExternalInput")
    weight = nc.dram_tensor("weight", [F_IN, F_OUT], FP32, kind="ExternalInput")
    a_vec = nc.dram_tensor("a_vec", [2 * F_OUT, 1], FP32, kind="ExternalInput")
    ident = nc.dram_tensor("ident", [128, 128], FP32, kind="ExternalInput")
    out_d = nc.dram_tensor("out", [ROWS, F_OUT], FP32, kind="ExternalOutput")

    with tile.TileContext(nc) as tc:
        for rep in range(repeat):
            _body(nc, tc, rep, A_blk, X_own, weight, a_vec, ident, out_d)

    nc.compile()
    return nc


def _body(nc, tc, rep, A_blk, X_own, weight, a_vec, ident, out_d):
    with tc.tile_pool(name=f"dram{rep}", bufs=1, space="DRAM") as dram_pool:
            A_bf = dram_pool.tile([ROWS, N], BF16)
            Xo_bf = dram_pool.tile([ROWS, F_IN], BF16)
            s_dram = dram_pool.tile([1, ROWS], FP32)
            r_dram = dram_pool.tile([1, ROWS], FP32)
            CCR = ROWS + 2 * RT  # wh rows + t rows (f32 as 2x bf16 rows)
            cc_in_m = nc.dram_tensor(f"cc_in_m{rep}", [CCR, F_OUT], BF16)
            cc_out_m = nc.dram_tensor(
                f"cc_out_m{rep}", [N_CORES * CCR, F_OUT], BF16,
                addr_space="Shared",
            )

            # ---- D2D casts (SWDGE): A int32 -> bf16, X f32 -> bf16 ----
            for c in range(DCH):
                w = N // DCH
                nc.gpsimd.dma_start(
                    out=A_bf[:, w * c : w * (c + 1)],
                    in_=A_blk[:, w * c : w * (c + 1)],
                )
            nc.gpsimd.dma_start(out=Xo_bf[:, :], in_=X_own[:, :])

            with (
                tc.tile_pool(name=f"setup{rep}", bufs=1) as setup,
                tc.tile_pool(name=f"whn{rep}", bufs=NT) as whn_pool,
                tc.tile_pool(name=f"slab{rep}", bufs=12) as slab_pool,
                tc.tile_pool(name=f"zz{rep}", bufs=4) as zz_pool,
                tc.tile_pool(name=f"pp{rep}", bufs=4) as pp_pool,
                tc.tile_pool(name=f"epi{rep}", bufs=2) as epi_pool,
                tc.tile_pool(name=f"psA{rep}", bufs=2, space="PSUM") as psA,
            ):
                # ---------------- setup ----------------
                idn = setup.tile([128, 128], FP32)
                nc.sync.dma_start(out=idn, in_=ident[:, :])
                ones_c = setup.tile([128, 1], BF16)
                nc.vector.memset(ones_c, 1.0)

                a_cat = setup.tile([128, 2], BF16)
                nc.gpsimd.dma_start(out=a_cat[:, 0:1], in_=a_vec[0:F_OUT, :])
                nc.gpsimd.dma_start(out=a_cat[:, 1:2], in_=a_vec[F_OUT:, :])

                # w_all[k] = [weight_k bf16 | w_t_k | w_s_k]  [128, 130]
                w_all = []
                for k in range(KC):
                    wa = setup.tile([128, F_OUT + 2], BF16, tag=f"w_all{k}")
                    nc.gpsimd.dma_start(
                        out=wa[:, 0:F_OUT], in_=weight[128 * k : 128 * (k + 1), :]
                    )
                    w_all.append(wa)
                for k in range(KC):
                    wT = setup.tile([128, 128], BF16, tag=f"wT{k}")
                    nc.sync.dma_start(
                        out=wT, in_=w_all[k][:, 0:F_OUT], transpose=True
                    )
                    ps = psA.tile([128, 2], FP32, tag="ps")
                    nc.tensor.matmul(ps, wT, a_cat, start=True, stop=True)
                    # col F_OUT = w_t (a[F:]), col F_OUT+1 = w_s (a[:F])
                    nc.vector.tensor_copy(
                        w_all[k][:, F_OUT : F_OUT + 1], ps[:, 1:2]
                    )
                    nc.vector.tensor_copy(
                        w_all[k][:, F_OUT + 1 : F_OUT + 2], ps[:, 0:1]
                    )

                # ---------------- X_own^T chunks ----------------
                xoT = []
                for k in range(KC):
                    xt = setup.tile([128, ROWS], BF16, tag=f"xoT{k}")
                    nc.sync.dma_start(
                        out=xt, in_=Xo_bf[:, 128 * k : 128 * (k + 1)], transpose=True
                    )
                    xoT.append(xt)

                # ------- own rows: [Wh | t | s] = Xo^T.T-contract @ w_all -------
                s_cols = setup.tile([128, RT], FP32)
                t_own = setup.tile([128, RT], FP32)
                for q in range(RT):
                    ps = psA.tile([128, F_OUT + 2], FP32, tag="ps")
                    for k in range(KC):
                        nc.tensor.matmul(
                            ps,
                            xoT[k][:, 128 * q : 128 * (q + 1)],
                            w_all[k],
                            start=(k == 0),
                            stop=(k == KC - 1),
                        )
                    wh = epi_pool.tile([128, F_OUT], BF16, tag="wh_own")
                    nc.vector.tensor_copy(wh, ps[:, 0:F_OUT])
                    nc.sync.dma_start(
                        out=cc_in_m[128 * q : 128 * (q + 1), :], in_=wh
                    )
                    nc.vector.tensor_copy(
                        t_own[:, q : q + 1], ps[:, F_OUT : F_OUT + 1]
                    )
                    nc.vector.tensor_copy(
                        s_cols[:, q : q + 1], ps[:, F_OUT + 1 : F_OUT + 2]
                    )
                # s -> DRAM row -> broadcast
                ps_sT = psA.tile([RT, 128], FP32, tag="ps")
                nc.tensor.transpose(ps_sT, s_cols, idn)
                sT = setup.tile([RT, 128], FP32)
                nc.vector.tensor_copy(sT, ps_sT)
                nc.sync.dma_start(out=s_dram[:, :], in_=sT)
                s_bc = setup.tile([128, ROWS], FP32)
                nc.gpsimd.dma_start(
                    out=s_bc,
                    in_=bass.AP(
                        tensor=s_dram.tensor, offset=s_dram.offset,
                        ap=[[0, 128], [1, ROWS]],
                    ),
                )
                # t-own -> [RT, 128] tile-major, f32 bitcast into bf16 rows
                ps_tT = psA.tile([RT, 128], FP32, tag="ps")
                nc.tensor.transpose(ps_tT, t_own, idn)
                tT = setup.tile([RT, 128], FP32)
                nc.vector.tensor_copy(tT, ps_tT)
                nc.sync.dma_start(
                    out=cc_in_m[ROWS : ROWS + 2 * RT, :],
                    in_=tT.bitcast(BF16),
                )

                # ------- single AllGather (Wh | t) across cores -------
                nc.gpsimd.collective_compute(
                    "AllGather", Alu.bypass,
                    replica_groups=[list(range(N_CORES))],
                    ins=[cc_in_m[:, :]], outs=[cc_out_m[:, :]],
                )
                # one big DMA for all Wh tiles: [128, NT, F_OUT]
                wh_all = setup.tile([128, NT, F_OUT], BF16)
                for c in range(N_CORES):
                    nc.sync.dma_start(
                        out=wh_all[:, RT * c : RT * (c + 1), :],
                        in_=cc_out_m[CCR * c : CCR * c + ROWS, :].rearrange(
                            "(r p) f -> p r f", p=128
                        ),
                    )
                wh_nat = [wh_all[:, r, :] for r in range(NT)]
                # t blocks: per core, 2*RT bf16 rows = [RT, 128] f32
                tg = setup.tile([NT, 128], FP32)
                for c in range(N_CORES):
                    nc.sync.dma_start(
                        out=tg[RT * c : RT * (c + 1), :].bitcast(BF16),
                        in_=cc_out_m[CCR * c + ROWS : CCR * c + ROWS + 2 * RT, :],
                    )
                ps_tc = psA.tile([128, NT], FP32, tag="ps")
                nc.tensor.transpose(ps_tc, tg, idn[0:NT, 0:NT])
                t_cols = setup.tile([128, NT], FP32)
                nc.vector.tensor_copy(t_cols, ps_tc)

                # ---------------- A^T slabs + main loop ----------------
                with (
                    tc.tile_pool(name=f"psO{rep}", bufs=1, space="PSUM") as psO,
                    tc.tile_pool(name=f"psD{rep}", bufs=1, space="PSUM") as psD,
                ):
                    ps_oT = psO.tile([128, ROWS], FP32)
                    ps_d = psD.tile([1, ROWS], FP32)
                    for jt in range(NT):
                        at = slab_pool.tile([128, ROWS], BF16)
                        nc.sync.dma_start(
                            out=at, in_=A_bf[:, 128 * jt : 128 * (jt + 1)],
                            transpose=True,
                        )
                        z = zz_pool.tile([128, ROWS], BF16)
                        nc.scalar.activation(
                            out=z, in_=s_bc, func=Act.Exp,
                            bias=t_cols[:, jt : jt + 1],
                        )
                        p = pp_pool.tile([128, ROWS], BF16)
                        nc.vector.scalar_tensor_tensor(
                            out=p, in0=z, scalar=1.0, in1=at,
                            op0=Alu.max, op1=Alu.mult,
                        )
                        first, last = jt == 0, jt == NT - 1
                        for h in range(2):
                            sl = slice(512 * h, 512 * (h + 1))
                            nc.tensor.matmul(
                                ps_oT[:, sl], wh_nat[jt], p[:, sl],
                                start=first, stop=last, skip_group_check=True,
                            )
                            nc.tensor.matmul(
                                ps_d[:, sl], ones_c, p[:, sl],
                                start=first, stop=last, skip_group_check=True,
                            )

                    # ---------------- epilogue ----------------
                    rec = epi_pool.tile([1, ROWS], FP32, tag="rec")
                    nc.vector.reciprocal(rec, ps_d)
                    nc.sync.dma_start(out=r_dram[:, :], in_=rec)
                    r_bc = epi_pool.tile([128, ROWS], FP32, tag="r_bc")
                    nc.gpsimd.dma_start(
                        out=r_bc,
                        in_=bass.AP(
                            tensor=r_dram.tensor, offset=r_dram.offset,
                            ap=[[0, 128], [1, ROWS]],
                        ),
                    )
                    xsc = epi_pool.tile([128, ROWS], FP32, tag="xsc")
                    nc.vector.tensor_tensor(
                        out=xsc, in0=ps_oT, in1=r_bc, op=Alu.mult
                    )
                    # ELU = exp(min(x,0)) - 1 + max(x,0)
                    m0 = epi_pool.tile([128, ROWS], FP32, tag="m0")
                    nc.vector.tensor_scalar(
                        out=m0, in0=xsc, scalar1=0.0, scalar2=None, op0=Alu.min
                    )
                    e0 = epi_pool.tile([128, ROWS], FP32, tag="e0")
                    nc.scalar.activation(out=e0, in_=m0, func=Act.Exp)
                    r0 = epi_pool.tile([128, ROWS], FP32, tag="r0")
                    nc.vector.tensor_scalar(
                        out=r0, in0=xsc, scalar1=0.0, scalar2=None, op0=Alu.max
                    )
                    oT = epi_pool.tile([128, ROWS], FP32, tag="oT")
                    nc.vector.scalar_tensor_tensor(
                        out=oT, in0=e0, scalar=-1.0, in1=r0,
                        op0=Alu.add, op1=Alu.add,
                    )
                    for q in range(RT):
                        ps_f = psA.tile([128, 128], FP32, tag="ps")
                        nc.tensor.transpose(
                            ps_f, oT[:, 128 * q : 128 * (q + 1)], idn
                        )
                        of = epi_pool.tile([128, F_OUT], FP32, tag="of")
                        nc.scalar.copy(of, ps_f)
                        nc.sync.dma_start(
                            out=out_d[128 * q : 128 * (q + 1), :], in_=of
                        )


def kernel(X, A, weight, a, _trace=False, _tmpdir=None):
    X = np.ascontiguousarray(np.asarray(X, dtype=np.float32))
    A = np.ascontiguousarray(np.asarray(A, dtype=np.int32))
    weight = np.ascontiguousarray(np.asarray(weight, dtype=np.float32))
    a = np.ascontiguousarray(np.asarray(a, dtype=np.float32))

    if "nc" not in _cache:
        _cache["nc"] = _build()
    nc = _cache["nc"]

    ident = np.eye(128, dtype=np.float32)
    in_maps = []
    for c in range(N_CORES):
        i0 = c * ROWS
        in_maps.append(
            {
                "A_blk": A[i0 : i0 + ROWS],
                "X_own": X[i0 : i0 + ROWS],
                "weight": weight,
                "a_vec": a,
                "ident": ident,
            }
        )

    res = run_bass_kernel_spmd(
        nc, in_maps, core_ids=list(range(N_CORES)), trace=_trace, tmpdir=_tmpdir
    )
    out = np.concatenate([res.results[c]["out"] for c in range(N_CORES)], axis=0)
    if _trace:
        kernel._last_results = res
    return out
